# revision 1
# baseline (speedup 1.0000x reference)
"""Circulant matmul kernel for Trainium2 (8 NeuronCores, SPMD).

Problem: out = input @ K + bias, where K[c, n] = weight[(c - n) mod 4096],
input is [1024, 4096] f32, weight/bias are [4096] f32.

Strategy (tensor-parallel / column-shard, per the sharding hint):
  - Host materializes X^T in bf16 (replicated to all 8 cores) and each
    core's 512-column slice of the circulant matrix K in bf16.
  - Core c computes out[:, 512c:512(c+1)] = X @ K_c + bias_c in fp32 PSUM.
    No collectives; host concatenates the 8 column slices.

Device kernel structure (per core):
  - xt chunks (32 x [128, 1024] bf16) DMA'd on the sync HWDGE queue,
    kc chunks (32 x [128, 512] bf16) on the scalar HWDGE queue, so
    descriptor generation is parallelized across both HW-DGE rings.
  - PE warm-up: full-width dummy matmuls on a DVE-memset scratch tile
    while the first input chunks land (lifts the HAM clock gate early).
  - Phase 1 processes chunks 0..23 across all 8 batch tiles (co-major,
    matches DMA arrival); phase 2 finishes each batch tile in turn
    (bt-major) so the bias-add + output DMA epilogues overlap the
    remaining matmuls.
"""

import numpy as np
import ml_dtypes

import concourse.bass as bass
import concourse.mybir as mybir
import concourse.tile as tile
from concourse import bacc
from concourse.bass import ts
from concourse.bass_utils import run_bass_kernel_spmd

N = 4096
BATCH = 1024
NCORES = 8
NSHARD = N // NCORES          # 512 output columns per core
P = 128                       # partitions
CO = N // P                   # 32 contraction chunks
BT = BATCH // P               # 8 batch tiles
CO_PH1 = CO - BT              # chunks processed co-major in phase 1

N_WARMUP = 9                  # full-width dummy matmuls to lift the HAM clock gate

BF16 = mybir.dt.bfloat16
F32 = mybir.dt.float32


def build_nc():
    """Build the per-core Bass program (same program on all cores; data differs)."""
    nc = bacc.Bacc("TRN2", target_bir_lowering=False, debug=False)

    xt_d = nc.dram_tensor("xt", [N, BATCH], BF16, kind="ExternalInput").ap()
    kc_d = nc.dram_tensor("kc", [N, NSHARD], BF16, kind="ExternalInput").ap()
    bias_d = nc.dram_tensor("biasb", [P, NSHARD], F32, kind="ExternalInput").ap()
    out_d = nc.dram_tensor("out", [BATCH, NSHARD], BF16, kind="ExternalOutput").ap()

    xt_r = xt_d.rearrange("(co ci) b -> ci co b", ci=P)      # [128, 32, 1024]
    kc_r = kc_d.rearrange("(co ci) n -> ci co n", ci=P)      # [128, 32, 512]

    with tile.TileContext(nc) as tc:
        with (
            tc.tile_pool(name="xpool", bufs=CO) as xpool,
            tc.tile_pool(name="kpool", bufs=CO) as kpool,
            tc.tile_pool(name="cpool", bufs=1) as cpool,
            tc.tile_pool(name="opool", bufs=4) as opool,
            tc.tile_pool(name="psum", bufs=BT, space="PSUM") as psum_pool,
        ):
            # scratch for PE warm-up, memset on the vector engine (fast start)
            scratch = cpool.tile([P, NSHARD], BF16, tag="scratch")
            nc.vector.memset(scratch[:], 0.125)

            # input streams: kc on scalar ring, xt on sync ring
            xt_tiles = []
            kc_tiles = []
            for co in range(CO):
                ktt = kpool.tile([P, NSHARD], BF16, tag="kc")
                nc.scalar.dma_start(ktt[:], kc_r[:, co, :])
                kc_tiles.append(ktt)
                xtt = xpool.tile([P, BATCH], BF16, tag="xt")
                nc.sync.dma_start(xtt[:], xt_r[:, co, :])
                xt_tiles.append(xtt)
            # bias last on the scalar ring: only needed for the epilogues
            bias_sb = cpool.tile([P, NSHARD], F32, tag="bias")
            nc.scalar.dma_start(bias_sb[:], bias_d)

            psum_tiles = [
                psum_pool.tile([P, NSHARD], F32, tag="ps", name=f"ps{bt}")
                for bt in range(BT)
            ]

            # PE warm-up: full 128-row dummy matmuls on scratch (HAM needs
            # real array activity; results are discarded by start=True below)
            for i in range(N_WARMUP):
                nc.tensor.matmul(
                    psum_tiles[i % BT][:],
                    scratch[:, :P],
                    scratch[:],
                    start=True,
                    stop=True,
                )

            # phase 1: chunks 0..CO_PH1-1, co-major (matches DMA arrival order)
            for co in range(CO_PH1):
                for bt in range(BT):
                    nc.tensor.matmul(
                        psum_tiles[bt][:],
                        xt_tiles[co][:, ts(bt, P)],   # lhsT [c=128, b=128]
                        kc_tiles[co][:],              # rhs  [c=128, n=512]
                        start=(co == 0),
                        stop=False,
                    )

            # phase 2: finish batch tiles one at a time; epilogue overlaps MMs
            for bt in range(BT):
                for co in range(CO_PH1, CO):
                    nc.tensor.matmul(
                        psum_tiles[bt][:],
                        xt_tiles[co][:, ts(bt, P)],
                        kc_tiles[co][:],
                        start=False,
                        stop=(co == CO - 1),
                    )
                out_sb = opool.tile([P, NSHARD], BF16, tag="osb")
                nc.vector.tensor_add(out_sb[:], psum_tiles[bt][:], bias_sb[:])
                nc.sync.dma_start(out_d[ts(bt, P), :], out_sb[:])

    nc.compile()
    return nc


def prepare_in_maps(input, weight, bias):
    x = np.asarray(input, dtype=np.float32)
    w = np.asarray(weight, dtype=np.float32)
    b = np.asarray(bias, dtype=np.float32)

    xt = np.ascontiguousarray(x.T).astype(ml_dtypes.bfloat16)   # [4096, 1024]

    c = np.arange(N)
    in_maps = []
    for core in range(NCORES):
        n0 = core * NSHARD
        idx = (c[:, None] - (n0 + np.arange(NSHARD))[None, :]) % N
        kc = w[idx].astype(ml_dtypes.bfloat16)                  # [4096, 512]
        bias_tile = np.ascontiguousarray(
            np.broadcast_to(b[n0 : n0 + NSHARD].astype(np.float32), (P, NSHARD))
        )
        in_maps.append({"xt": xt, "kc": kc, "biasb": bias_tile})
    return in_maps


_NC_CACHE = None


def _get_nc():
    global _NC_CACHE
    if _NC_CACHE is None:
        _NC_CACHE = build_nc()
    return _NC_CACHE


def kernel(**inputs):
    nc = _get_nc()
    in_maps = prepare_in_maps(inputs["input"], inputs["weight"], inputs["bias"])
    res = run_bass_kernel_spmd(nc, in_maps, list(range(NCORES)))
    out = np.empty((BATCH, N), dtype=np.float32)
    for core in range(NCORES):
        out[:, core * NSHARD : (core + 1) * NSHARD] = res.results[core]["out"].astype(
            np.float32
        )
    return out



# revision 3
# speedup vs baseline: 2.6943x; 2.6943x over previous
"""Circulant matmul kernel for Trainium2 (8 NeuronCores, SPMD) via real CRT.

Problem: out = input @ K + bias, K[i, j] = weight[(i - j) mod 4096],
input [1024, 4096] f32, weight/bias [4096] f32.

Algorithm: out_b = x_b (cyclic-conv) v with v[m] = w[(-m) mod n].  Over the
reals, z^4096 - 1 factors into 8 degree-512 polynomials (z^512 -+ 1 and
trinomials z^512 - g z^256 + 1, g = 2cos(theta)).  The host reduces x mod
each factor (O(n log n) shift-adds, part of sharding prep), core k multiplies
its residue by the fixed multiplication matrix M_k of v in its ring -- a
dense [1024, 512] @ [512, 512] matmul, 8x fewer MACs than the dense
circulant -- and the host interpolates the 8 results back (O(n) ladder) and
adds bias.  No collectives; each core's inputs are unique (no replication).

Device kernel per core:
  - xt chunks (4 x [128, 1024] bf16) on the sync HWDGE queue, M chunks
    (4 x [128, 512] bf16) on the scalar queue.
  - PE warm-up dummy matmuls lift the HAM clock gate while DMAs land.
  - Phase 1: chunks 0..2 co-major across 8 batch tiles; phase 2 finishes
    each batch tile (chunk 3) and streams its PSUM->SBUF copy (alternating
    vector/scalar engines) + output DMA so epilogues overlap the tail.
"""

import numpy as np
import ml_dtypes

import concourse.bass as bass
import concourse.mybir as mybir
import concourse.tile as tile
from concourse import bacc
from concourse.bass import ts
from concourse.bass_utils import run_bass_kernel_spmd

N = 4096
BATCH = 1024
NCORES = 8
L = 3                         # CRT levels -> 8 factors of degree 512
DEG = N >> L                  # 512
P = 128
CO = DEG // P                 # 4 contraction chunks
BT = BATCH // P               # 8 batch tiles
CO_PH1 = CO - 1               # chunks done co-major in phase 1

N_WARMUP = 6

BF16 = mybir.dt.bfloat16
F32 = mybir.dt.float32


# ---------------- CRT factor tree (host side) ----------------
# node ('pm', k, s): z^k - s ;  node ('tri', k, g): z^k - g z^(k/2) + 1

def _children(node):
    kind = node[0]
    if kind == 'pm':
        _, k, s = node
        if s == +1:
            return [('pm', k // 2, +1), ('pm', k // 2, -1)]
        return [('tri', k // 2, np.sqrt(2.0)), ('tri', k // 2, -np.sqrt(2.0))]
    _, k, g = node
    a = 2.0 * np.cos(np.arccos(g / 2.0) / 2.0)
    return [('tri', k // 2, a), ('tri', k // 2, -a)]


def _reduce_child(r, child):
    k = r.shape[-1] // 2
    r0, r1 = r[..., :k], r[..., k:]
    if child[0] == 'pm':
        return r0 + child[2] * r1
    g = child[2]
    h = k // 2
    r1a, r1b = r1[..., :h], r1[..., h:]
    res = (r0 - r1).copy()
    res[..., :h] -= g * r1b
    res[..., h:] += g * (r1a + g * r1b)
    return res


def _reduce_tree(x, node, lvl):
    if lvl == 0:
        return [x]
    out = []
    for ch in _children(node):
        out += _reduce_tree(_reduce_child(x, ch), ch, lvl - 1)
    return out


def _interp_pair(rA, rB, chA, chB):
    if chA[0] == 'pm':
        y0 = 0.5 * (rA + rB)
        y1 = 0.5 * (rA - rB)
        return np.concatenate([y0, y1], axis=-1)
    a = chA[2]
    h = rA.shape[-1] // 2
    rAlo, rAhi = rA[..., :h], rA[..., h:]
    rBlo, rBhi = rB[..., :h], rB[..., h:]
    s = (rAlo - rBlo) / (-2.0 * a)
    rr = (rAhi - rBhi) / (2.0 * a)
    p = rAlo + rr + a * s
    q = rAhi - a * rr - (a * a - 1.0) * s
    return np.concatenate([p, q, rr, s], axis=-1)


def _interp_tree(res_list, node, lvl):
    if lvl == 0:
        return res_list[0]
    chA, chB = _children(node)
    half = len(res_list) // 2
    return _interp_pair(
        _interp_tree(res_list[:half], chA, lvl - 1),
        _interp_tree(res_list[half:], chB, lvl - 1),
        chA, chB,
    )


_ROOT = ('pm', N, +1)


def _factors(node, lvl):
    if lvl == 0:
        return [node]
    return [f for ch in _children(node) for f in _factors(ch, lvl - 1)]


FACTORS = _factors(_ROOT, L)


def _mult_matrix(v_res, fac):
    """M[i, j] = coeff j of z^i * v(z) mod F."""
    k = v_res.shape[-1]
    M = np.zeros((k, k))
    row = v_res.astype(np.float64).copy()
    for i in range(k):
        M[i] = row
        top = row[-1]
        row[1:] = row[:-1]
        row[0] = 0.0
        if fac[0] == 'pm':
            row[0] += fac[2] * top
        else:
            row[0] -= top
            row[k // 2] += fac[2] * top
    return M


# ---------------- device kernel ----------------

def build_nc():
    nc = bacc.Bacc("TRN2", target_bir_lowering=False, debug=False)

    xt_d = nc.dram_tensor("xt", [DEG, BATCH], BF16, kind="ExternalInput").ap()
    m_d = nc.dram_tensor("mk", [DEG, DEG], BF16, kind="ExternalInput").ap()
    out_d = nc.dram_tensor("out", [BATCH, DEG], BF16, kind="ExternalOutput").ap()

    xt_r = xt_d.rearrange("(co ci) b -> ci co b", ci=P)   # [128, 4, 1024]
    m_r = m_d.rearrange("(co ci) n -> ci co n", ci=P)     # [128, 4, 512]

    with tile.TileContext(nc) as tc:
        with (
            tc.tile_pool(name="xpool", bufs=CO) as xpool,
            tc.tile_pool(name="mpool", bufs=CO) as mpool,
            tc.tile_pool(name="cpool", bufs=1) as cpool,
            tc.tile_pool(name="opool", bufs=BT) as opool,
            tc.tile_pool(name="psum", bufs=BT, space="PSUM") as psum_pool,
        ):
            scratch = cpool.tile([P, DEG], BF16, tag="scratch")
            nc.vector.memset(scratch[:], 0.125)

            xt_tiles = []
            m_tiles = []
            for co in range(CO):
                mtt = mpool.tile([P, DEG], BF16, tag="mk")
                nc.scalar.dma_start(mtt[:], m_r[:, co, :])
                m_tiles.append(mtt)
                xtt = xpool.tile([P, BATCH], BF16, tag="xt")
                nc.sync.dma_start(xtt[:], xt_r[:, co, :])
                xt_tiles.append(xtt)

            psum_tiles = [
                psum_pool.tile([P, DEG], F32, tag="ps", name=f"ps{bt}")
                for bt in range(BT)
            ]

            # PE warm-up on scratch (HAM clock gate)
            for i in range(N_WARMUP):
                nc.tensor.matmul(
                    psum_tiles[i % BT][:],
                    scratch[:, :P],
                    scratch[:],
                    start=True,
                    stop=True,
                )

            # phase 1: chunks 0..CO_PH1-1 co-major (matches DMA arrival)
            for co in range(CO_PH1):
                for bt in range(BT):
                    nc.tensor.matmul(
                        psum_tiles[bt][:],
                        xt_tiles[co][:, ts(bt, P)],
                        m_tiles[co][:],
                        start=(co == 0),
                        stop=False,
                    )

            # phase 2: last chunk per batch tile + epilogue, engines alternate
            for bt in range(BT):
                nc.tensor.matmul(
                    psum_tiles[bt][:],
                    xt_tiles[CO - 1][:, ts(bt, P)],
                    m_tiles[CO - 1][:],
                    start=False,
                    stop=True,
                )
                out_sb = opool.tile([P, DEG], BF16, tag="osb")
                if bt % 2 == 0:
                    nc.vector.tensor_scalar_mul(out_sb[:], psum_tiles[bt][:], 1.0)
                else:
                    nc.scalar.copy(out_sb[:], psum_tiles[bt][:])
                nc.sync.dma_start(out_d[ts(bt, P), :], out_sb[:])

    nc.compile()
    return nc


def prepare_in_maps(input, weight, bias):
    x = np.asarray(input, dtype=np.float64)
    w = np.asarray(weight, dtype=np.float64)
    v = np.roll(w[::-1], 1)                      # v[m] = w[(-m) mod n]

    x_res = _reduce_tree(x, _ROOT, L)            # 8 x [1024, 512]
    v_res = _reduce_tree(v, _ROOT, L)            # 8 x [512]

    in_maps = []
    for core in range(NCORES):
        xt = np.ascontiguousarray(x_res[core].T).astype(ml_dtypes.bfloat16)
        M = _mult_matrix(v_res[core], FACTORS[core]).astype(ml_dtypes.bfloat16)
        in_maps.append({"xt": xt, "mk": M})
    return in_maps


def finish(results, bias):
    y_res = [results[c]["out"].astype(np.float64) for c in range(NCORES)]
    y = _interp_tree(y_res, _ROOT, L)
    return (y + np.asarray(bias, np.float64)).astype(np.float32)


_NC_CACHE = None


def _get_nc():
    global _NC_CACHE
    if _NC_CACHE is None:
        _NC_CACHE = build_nc()
    return _NC_CACHE


def kernel(**inputs):
    nc = _get_nc()
    in_maps = prepare_in_maps(inputs["input"], inputs["weight"], inputs["bias"])
    res = run_bass_kernel_spmd(nc, in_maps, list(range(NCORES)))
    return finish(res.results, inputs["bias"])


# revision 5
# speedup vs baseline: 2.7799x; 1.0318x over previous
"""Circulant matmul kernel for Trainium2 (8 NeuronCores, SPMD) via real CRT.

Problem: out = input @ K + bias, K[i, j] = weight[(i - j) mod 4096],
input [1024, 4096] f32, weight/bias [4096] f32.

Algorithm: out_b = x_b (cyclic-conv) v with v[m] = w[(-m) mod n].  Over the
reals, z^4096 - 1 factors into 8 degree-512 polynomials (z^512 -+ 1 and
trinomials z^512 - g z^256 + 1, g = 2cos(theta)).  The host reduces x mod
each factor (O(n log n) shift-adds, part of sharding prep), core k multiplies
its residue by the fixed multiplication matrix M_k of v in its ring -- a
dense [1024, 512] @ [512, 512] matmul, 8x fewer MACs than the dense
circulant -- and the host interpolates the 8 results back (O(n) ladder) and
adds bias.  No collectives; each core's inputs are unique (no replication).

Device kernel per core:
  - xt chunks (4 x [128, 1024] bf16) on the sync HWDGE queue, M chunks
    (4 x [128, 512] bf16) on the scalar queue.
  - PE warm-up dummy matmuls lift the HAM clock gate while DMAs land.
  - Phase 1: chunks 0..2 co-major across 8 batch tiles; phase 2 finishes
    each batch tile (chunk 3) and streams its PSUM->SBUF copy (alternating
    vector/scalar engines) + output DMA so epilogues overlap the tail.
"""

import numpy as np
import ml_dtypes

import concourse.bass as bass
import concourse.mybir as mybir
import concourse.tile as tile
from concourse import bacc
from concourse.bass import ts
from concourse.bass_utils import run_bass_kernel_spmd

N = 4096
BATCH = 1024
NCORES = 8
L = 3                         # CRT levels -> 8 factors of degree 512
DEG = N >> L                  # 512
P = 128
CO = DEG // P                 # 4 contraction chunks
BT = BATCH // P               # 8 batch tiles
CO_PH1 = CO - 2               # chunks done co-major in phase 1

N_WARMUP = 12                 # narrow warm-up matmuls (HAM clock gate + DMA wait)
WARM_W = 128

BF16 = mybir.dt.bfloat16
F32 = mybir.dt.float32


# ---------------- CRT factor tree (host side) ----------------
# node ('pm', k, s): z^k - s ;  node ('tri', k, g): z^k - g z^(k/2) + 1

def _children(node):
    kind = node[0]
    if kind == 'pm':
        _, k, s = node
        if s == +1:
            return [('pm', k // 2, +1), ('pm', k // 2, -1)]
        return [('tri', k // 2, np.sqrt(2.0)), ('tri', k // 2, -np.sqrt(2.0))]
    _, k, g = node
    a = 2.0 * np.cos(np.arccos(g / 2.0) / 2.0)
    return [('tri', k // 2, a), ('tri', k // 2, -a)]


def _reduce_child(r, child):
    k = r.shape[-1] // 2
    r0, r1 = r[..., :k], r[..., k:]
    if child[0] == 'pm':
        return r0 + child[2] * r1
    g = child[2]
    h = k // 2
    r1a, r1b = r1[..., :h], r1[..., h:]
    res = (r0 - r1).copy()
    res[..., :h] -= g * r1b
    res[..., h:] += g * (r1a + g * r1b)
    return res


def _reduce_tree(x, node, lvl):
    if lvl == 0:
        return [x]
    out = []
    for ch in _children(node):
        out += _reduce_tree(_reduce_child(x, ch), ch, lvl - 1)
    return out


def _interp_pair(rA, rB, chA, chB):
    if chA[0] == 'pm':
        y0 = 0.5 * (rA + rB)
        y1 = 0.5 * (rA - rB)
        return np.concatenate([y0, y1], axis=-1)
    a = chA[2]
    h = rA.shape[-1] // 2
    rAlo, rAhi = rA[..., :h], rA[..., h:]
    rBlo, rBhi = rB[..., :h], rB[..., h:]
    s = (rAlo - rBlo) / (-2.0 * a)
    rr = (rAhi - rBhi) / (2.0 * a)
    p = rAlo + rr + a * s
    q = rAhi - a * rr - (a * a - 1.0) * s
    return np.concatenate([p, q, rr, s], axis=-1)


def _interp_tree(res_list, node, lvl):
    if lvl == 0:
        return res_list[0]
    chA, chB = _children(node)
    half = len(res_list) // 2
    return _interp_pair(
        _interp_tree(res_list[:half], chA, lvl - 1),
        _interp_tree(res_list[half:], chB, lvl - 1),
        chA, chB,
    )


_ROOT = ('pm', N, +1)


def _factors(node, lvl):
    if lvl == 0:
        return [node]
    return [f for ch in _children(node) for f in _factors(ch, lvl - 1)]


FACTORS = _factors(_ROOT, L)


def _mult_matrix(v_res, fac):
    """M[i, j] = coeff j of z^i * v(z) mod F."""
    k = v_res.shape[-1]
    M = np.zeros((k, k))
    row = v_res.astype(np.float64).copy()
    for i in range(k):
        M[i] = row
        top = row[-1]
        row[1:] = row[:-1]
        row[0] = 0.0
        if fac[0] == 'pm':
            row[0] += fac[2] * top
        else:
            row[0] -= top
            row[k // 2] += fac[2] * top
    return M


# ---------------- device kernel ----------------

def build_nc():
    nc = bacc.Bacc("TRN2", target_bir_lowering=False, debug=False)

    xt_d = nc.dram_tensor("xt", [DEG, BATCH], BF16, kind="ExternalInput").ap()
    m_d = nc.dram_tensor("mk", [DEG, DEG], BF16, kind="ExternalInput").ap()
    out_d = nc.dram_tensor("out", [BATCH, DEG], BF16, kind="ExternalOutput").ap()

    xt_r = xt_d.rearrange("(co ci) b -> ci co b", ci=P)   # [128, 4, 1024]
    m_r = m_d.rearrange("(co ci) n -> ci co n", ci=P)     # [128, 4, 512]

    with tile.TileContext(nc) as tc:
        with (
            tc.tile_pool(name="xpool", bufs=CO) as xpool,
            tc.tile_pool(name="mpool", bufs=CO) as mpool,
            tc.tile_pool(name="cpool", bufs=1) as cpool,
            tc.tile_pool(name="opool", bufs=BT) as opool,
            tc.tile_pool(name="psum", bufs=BT, space="PSUM") as psum_pool,
        ):
            scratch = cpool.tile([P, DEG], BF16, tag="scratch")
            nc.vector.memset(scratch[:], 0.125)

            xt_tiles = []
            m_tiles = []
            for co in range(CO):
                mtt = mpool.tile([P, DEG], BF16, tag="mk")
                nc.scalar.dma_start(mtt[:], m_r[:, co, :])
                m_tiles.append(mtt)
                xtt = xpool.tile([P, BATCH], BF16, tag="xt")
                nc.sync.dma_start(xtt[:], xt_r[:, co, :])
                xt_tiles.append(xtt)

            psum_tiles = [
                psum_pool.tile([P, DEG], F32, tag="ps", name=f"ps{bt}")
                for bt in range(BT)
            ]

            # PE warm-up on scratch (HAM clock gate); narrow so little PE
            # time is wasted at the ramped-down clock
            for i in range(N_WARMUP):
                nc.tensor.matmul(
                    psum_tiles[i % BT][:, :WARM_W],
                    scratch[:, :P],
                    scratch[:, :WARM_W],
                    start=True,
                    stop=True,
                )

            # phase 1: chunks 0..CO_PH1-1 co-major (matches DMA arrival)
            for co in range(CO_PH1):
                for bt in range(BT):
                    nc.tensor.matmul(
                        psum_tiles[bt][:],
                        xt_tiles[co][:, ts(bt, P)],
                        m_tiles[co][:],
                        start=(co == 0),
                        stop=False,
                    )

            # phase 2: finish each batch tile (bt-major) so its epilogue --
            # PSUM->SBUF copy (alternating vector/scalar) + output DMA
            # (alternating sync/scalar rings) -- overlaps remaining matmuls
            for bt in range(BT):
                for co in range(CO_PH1, CO):
                    nc.tensor.matmul(
                        psum_tiles[bt][:],
                        xt_tiles[co][:, ts(bt, P)],
                        m_tiles[co][:],
                        start=False,
                        stop=(co == CO - 1),
                    )
                out_sb = opool.tile([P, DEG], BF16, tag="osb")
                if bt % 2 == 0:
                    nc.vector.tensor_scalar_mul(out_sb[:], psum_tiles[bt][:], 1.0)
                else:
                    nc.scalar.copy(out_sb[:], psum_tiles[bt][:])
                ring = nc.sync if bt % 2 == 0 else nc.scalar
                ring.dma_start(out_d[ts(bt, P), :], out_sb[:])

    nc.compile()
    return nc


def prepare_in_maps(input, weight, bias):
    x = np.asarray(input, dtype=np.float64)
    w = np.asarray(weight, dtype=np.float64)
    v = np.roll(w[::-1], 1)                      # v[m] = w[(-m) mod n]

    x_res = _reduce_tree(x, _ROOT, L)            # 8 x [1024, 512]
    v_res = _reduce_tree(v, _ROOT, L)            # 8 x [512]

    in_maps = []
    for core in range(NCORES):
        xt = np.ascontiguousarray(x_res[core].T).astype(ml_dtypes.bfloat16)
        M = _mult_matrix(v_res[core], FACTORS[core]).astype(ml_dtypes.bfloat16)
        in_maps.append({"xt": xt, "mk": M})
    return in_maps


def finish(results, bias):
    y_res = [results[c]["out"].astype(np.float64) for c in range(NCORES)]
    y = _interp_tree(y_res, _ROOT, L)
    return (y + np.asarray(bias, np.float64)).astype(np.float32)


_NC_CACHE = None


def _get_nc():
    global _NC_CACHE
    if _NC_CACHE is None:
        _NC_CACHE = build_nc()
    return _NC_CACHE


def kernel(**inputs):
    nc = _get_nc()
    in_maps = prepare_in_maps(inputs["input"], inputs["weight"], inputs["bias"])
    res = run_bass_kernel_spmd(nc, in_maps, list(range(NCORES)))
    return finish(res.results, inputs["bias"])
